# revision 18
# baseline (speedup 1.0000x reference)
"""Trainium2 Bass kernel for nn_AttentionFlowLayer (trilinear similarity).

Reference math (per batch b):
    S[t, j] = (H[t] * w3) . U[j]  +  H[t] . w1  +  U[j] . w2

Folded form used here: with U'[j, d] = w3[d] * U[j, d] + w1[d] and
s_u[j] = U[j] . w2,

    S^T[j, t] = sum_d U'[j, d] * H[t, d]  +  s_u[j]

so each 128x512 output tile of S^T needs ONE fp16 matmul
(lhsT = U'^T chunk, rhs = H^T chunk) and the s_u bias is per-partition,
folded for free into the PSUM->SBUF copy. The kernel computes and
stores in fp16 (inputs cast f32->fp16 on-chip, output S^T stored fp16,
host upconverts): halves HBM write traffic (16 -> 8 MiB) and cuts PE
cost (fp16 LDWEIGHTS loads 2 elem/cycle, fp16 transposes 1 cyc/row) vs
the f32r baseline, with rel-err ~5e-4 against the f32 reference.

Work split per engine (GPSIMD cannot read PSUM, so PSUM drains are
scalar/vector only):
  PE:      w-prep rank-1s, 32 fp16 128x128 transposes, 64 fp16 N=512 matmuls
  Scalar:  H fp16 cast, H^T copies, even-row output copies (+s_u bias)
  Vector:  U'^T scale-copies (w3*x+w1), s_u row-dots, odd-row output copies
  GpSimd:  U fp16 cast, identity
  Sync:    all HBM DMAs (queue): inputs + 16x 512KiB output writes

Sharding: data-parallel over batch - 8 batches, one per NeuronCore.
Self-contained: hardcodes shapes B=8, T=J=2048, D=128.
"""

import numpy as np

import concourse.mybir as mybir
import concourse.tile as tile
from concourse import bacc
from concourse.bass import broadcast_tensor_aps
from concourse.bass_utils import run_bass_kernel_spmd
from concourse.masks import make_identity

F32 = mybir.dt.float32
F16 = mybir.dt.float16
IDENT = mybir.ActivationFunctionType.Identity
MULT = mybir.AluOpType.mult
ADD = mybir.AluOpType.add

B = 8          # batch -> one per core
T = 2048       # rows of S (t) and columns (j)
D = 128        # feature dim = contraction K
P = 128        # partitions / tile edge
NT = T // P    # 16 tiles per side
MMW = 512      # matmul moving width (1 PSUM bank of f32 out)
NH = T // MMW  # 4 chunks per output row-block

U_LEAD = 3     # U-transpose lead distance ahead of the main loop

_NC_CACHE = {}


def _build_nc():
    nc = bacc.Bacc(
        "TRN2",
        target_bir_lowering=False,
        debug=False,
        num_devices=B,
    )
    H = nc.dram_tensor("H", [T, D], F32, kind="ExternalInput").ap()
    U = nc.dram_tensor("U", [T, D], F32, kind="ExternalInput").ap()
    w = nc.dram_tensor("weight", [3 * D], F32, kind="ExternalInput").ap()
    # Holds S^T (fp16) for this batch; host transposes + upcasts on gather.
    S = nc.dram_tensor("S", [T, T], F16, kind="ExternalOutput").ap()

    with tile.TileContext(nc) as tc:
        with (
            tc.tile_pool(name="persist", bufs=1) as pp,
            tc.tile_pool(name="psum_mm", bufs=6, space="PSUM") as psum_mm,
            tc.tile_pool(name="psum_tr", bufs=2, space="PSUM") as psum_tr,
            tc.tile_pool(name="outp", bufs=6) as outp,
        ):
            ident16 = pp.tile([P, P], F16)
            make_identity(nc, ident16[:])

            # Inputs, natural layout [p, ti, d] (t = ti*128 + p) for H;
            # U loads p-major (fully contiguous per partition -> fast DMA).
            # U_sb[p, k, d] = U[16p + k, d]: j-tile k covers j = 16q + k,
            # a row permutation absorbed by the output DMA access pattern.
            H_sb = pp.tile([P, NT, D], F32)
            U_sb = pp.tile([P, NT, D], F32)
            H16 = pp.tile([P, NT, D], F16)
            U16 = pp.tile([P, NT, D], F16)
            H_r = H.rearrange("(ti p) d -> p ti d", p=P)
            U_r = U.rearrange("(p k) d -> p k d", p=P)
            # weight first, on its own queue: it gates the w-prep matmuls.
            w_row = pp.tile([1, 3 * D], F32)
            nc.gpsimd.dma_start(out=w_row[:], in_=w.unsqueeze(0))
            # H leads (it gates the H transposes -> the first matmuls);
            # U's first quarter early so U transposes k=0..2 can start.
            nc.sync.dma_start(out=H_sb[:, 0:4, :], in_=H_r[:, 0:4, :])
            nc.sync.dma_start(out=H_sb[:, 4:8, :], in_=H_r[:, 4:8, :])
            nc.sync.dma_start(out=U_sb[:, 0:4, :], in_=U_r[:, 0:4, :])
            nc.sync.dma_start(out=H_sb[:, 8:12, :], in_=H_r[:, 8:12, :])
            nc.sync.dma_start(out=H_sb[:, 12:16, :], in_=H_r[:, 12:16, :])
            nc.sync.dma_start(out=U_sb[:, 4:8, :], in_=U_r[:, 4:8, :])
            nc.sync.dma_start(out=U_sb[:, 8:16, :], in_=U_r[:, 8:16, :])

            one_cell = pp.tile([1, 1], F32)
            nc.vector.memset(one_cell[:], 1.0)
            ones_row = pp.tile([1, P], F32)
            nc.vector.memset(ones_row[:], 1.0)

            # w1/w3 columns [d, 1] via rank-1 matmuls (w_row_chunk^T x 1)
            wc_b = psum_mm.tile([P, MMW], F32, tag="mm", name="wc_b")
            wcol_ps = wc_b[:, 0:2]
            for i, k in enumerate((0, 2)):
                nc.tensor.matmul(
                    wcol_ps[:, i : i + 1],
                    w_row[0:1, k * D : (k + 1) * D],
                    one_cell[:],
                    start=True,
                    stop=True,
                )
            wcol = pp.tile([P, 2], F32)
            nc.scalar.copy(wcol[:], wcol_ps[:])
            w1col = wcol[:, 0:1]
            w3col = wcol[:, 1:2]
            # w2 broadcast to all partitions: w2b[p, d] = w2[d], via
            # ones-column (K=1) matmul. Feeds the s_u row-dots on DVE.
            w2_b = psum_mm.tile([P, MMW], F32, tag="mm", name="w2_b")
            w2_ps = w2_b[:, 0:D]
            nc.tensor.matmul(
                w2_ps[:], ones_row[:], w_row[0:1, D : 2 * D], start=True, stop=True
            )
            w2b = pp.tile([P, D], F32)
            nc.scalar.copy(w2b[:], w2_ps[:])

            # fp16 casts, chunked so transposes can start as DMAs land.
            # H on scalar (gates transposes), U on gpsimd (otherwise idle).
            def cast_h(c):
                csl = slice(4 * c, 4 * c + 4)
                nc.scalar.copy(H16[:, csl, :], H_sb[:, csl, :])

            def cast_u(c):
                csl = slice(4 * c, 4 * c + 4)
                nc.gpsimd.tensor_copy(U16[:, csl, :], U_sb[:, csl, :])

            # Persistent transposed operands (d on partitions), fp16
            HT = pp.tile([P, T], F16)      # H^T
            UpT = pp.tile([P, T], F16)     # U'^T = w3 * U^T + w1
            s_u_col = pp.tile([P, NT], F32)  # s_u, one col per j-tile
            suprod = pp.tile([P, 4, D], F32)  # U * w2 scratch, one 4-chunk

            def do_h(ti):
                csl = slice(ti * P, (ti + 1) * P)
                h_ps = psum_tr.tile([P, P], F16, tag="tr", name=f"h_ps{ti}")
                nc.tensor.transpose(h_ps[:], H16[:, ti, :], ident16[:])
                nc.scalar.copy(HT[:, csl], h_ps[:])

            def do_u(k):
                csl = slice(k * P, (k + 1) * P)
                u_ps = psum_tr.tile([P, P], F16, tag="tr", name=f"u_ps{k}")
                nc.tensor.transpose(u_ps[:], U16[:, k, :], ident16[:])
                # U'^T chunk = w3 * U^T + w1, rounded to fp16
                nc.vector.tensor_scalar(
                    UpT[:, csl], u_ps[:], w3col, w1col, op0=MULT, op1=ADD
                )

            def do_su(c):
                # s_u[16p+k] = sum_d U_sb[p,k,d] * w2[d] for k in one
                # 4-tile chunk, on DVE: broadcast-multiply then reduce-X.
                csl = slice(4 * c, 4 * c + 4)
                in0 = U_sb[:, csl, :]
                in1 = w2b[:].unsqueeze(1)  # [P, 1, D] -> bcast over k
                in0b, in1b = broadcast_tensor_aps(in0, in1)
                nc.vector.tensor_tensor(
                    out=suprod[:], in0=in0b, in1=in1b, op=MULT
                )
                nc.vector.tensor_reduce(
                    s_u_col[:, csl], suprod[:], axis=mybir.AxisListType.X, op=ADD
                )

            cast_h(0)
            cast_u(0)
            for ti in range(4):
                do_h(ti)
            for k in range(U_LEAD):
                do_u(k)
            do_su(0)
            cast_h(1)
            cast_u(1)

            # Main loop: one 128-row output block of S^T per jt, four
            # N=512 fp16 matmuls; the PSUM->SBUF copy folds the s_u bias
            # and the fp16 downcast. All 4 copies of a row-block go to ONE
            # engine (alternating per row) so each output DMA waits on a
            # single engine's semaphore; two rows in flight across the two
            # engines. H transposes/casts fold lazily into jt==0; s_u
            # row-dots spread 2-per-row over the early iterations.
            for jt in range(NT):
                if 1 <= jt <= 3:
                    do_su(jt)
                if jt + U_LEAD < NT:
                    do_u(jt + U_LEAD)
                jsl = slice(jt * P, (jt + 1) * P)
                S_rows = S.rearrange("(q s) t -> s q t", s=NT)[jt]
                su_b = s_u_col[:, jt : jt + 1]
                out_sb = outp.tile([P, T], F16)
                for h in range(NH):
                    if jt == 0 and h >= 1:
                        if h + 1 < 4:
                            cast_h(h + 1)
                        for ti in range(4 * h, 4 * h + 4):
                            do_h(ti)
                    if jt == 1 and h < 2:
                        cast_u(h + 2)
                    tsl = slice(h * MMW, (h + 1) * MMW)
                    ps = psum_mm.tile([P, MMW], F32, tag="mm", name=f"mm{jt}_{h}")
                    nc.tensor.matmul(
                        ps[:], UpT[:, jsl], HT[:, tsl], start=True, stop=True
                    )
                    if jt % 2 == 0:
                        nc.scalar.activation(
                            out_sb[:, tsl], ps[:], IDENT, bias=su_b, scale=1.0
                        )
                    else:
                        nc.vector.tensor_scalar_add(out_sb[:, tsl], ps[:], su_b)
                    # First row-block: quarters so output DMA starts early.
                    if jt == 0:
                        nc.sync.dma_start(out=S_rows[:, tsl], in_=out_sb[:, tsl])
                if jt > 0:
                    nc.sync.dma_start(out=S_rows[:, :], in_=out_sb[:])

    nc.compile()
    return nc


def _get_nc():
    if "nc" not in _NC_CACHE:
        _NC_CACHE["nc"] = _build_nc()
    return _NC_CACHE["nc"]


def kernel_with_results(H, U, weight, trace=False):
    assert H.shape == (B, T, D) and U.shape == (B, T, D)
    assert weight.shape == (3 * D,)
    nc = _get_nc()
    in_maps = [
        {
            "H": np.ascontiguousarray(H[b], dtype=np.float32),
            "U": np.ascontiguousarray(U[b], dtype=np.float32),
            "weight": np.ascontiguousarray(weight, dtype=np.float32),
        }
        for b in range(B)
    ]
    res = run_bass_kernel_spmd(nc, in_maps, list(range(B)), trace=trace)
    # device output is S^T (fp16) per batch
    out = np.stack(
        [np.asarray(res.results[b]["S"]).T.astype(np.float32) for b in range(B)],
        axis=0,
    )
    return out, res


def kernel(H, U, weight):
    out, _ = kernel_with_results(H, U, weight)
    return out


if __name__ == "__main__":
    rng = np.random.default_rng(0)
    H = rng.standard_normal((B, T, D)).astype(np.float32)
    U = rng.standard_normal((B, T, D)).astype(np.float32)
    w = rng.random(3 * D).astype(np.float32)
    out = kernel(H, U, w)
    print(out.shape, out.dtype)


# revision 23
# speedup vs baseline: 1.0445x; 1.0445x over previous
"""Trainium2 Bass kernel for nn_AttentionFlowLayer (trilinear similarity).

Reference math (per batch b):
    S[t, j] = (H[t] * w3) . U[j]  +  H[t] . w1  +  U[j] . w2

Folded form used here: with U'[j, d] = w3[d] * U[j, d] + w1[d] and
s_u[j] = U[j] . w2,

    S^T[j, t] = sum_d U'[j, d] * H[t, d]  +  s_u[j]

so each 128x512 output tile of S^T needs ONE fp16 matmul
(lhsT = U'^T chunk, rhs = H^T chunk) and the s_u bias is per-partition,
folded for free into the PSUM->SBUF copy. The kernel computes and
stores in fp16 (inputs cast f32->fp16 on-chip, output S^T stored fp16,
host upconverts): halves HBM write traffic (16 -> 8 MiB) and cuts PE
cost (fp16 LDWEIGHTS loads 2 elem/cycle, fp16 transposes 1 cyc/row) vs
the f32r baseline, with rel-err ~5e-4 against the f32 reference.

Work split per engine (GPSIMD cannot read PSUM, so PSUM drains are
scalar/vector only):
  PE:      w-prep rank-1s, 32 fp16 128x128 transposes, 64 fp16 N=512 matmuls
  Scalar:  H fp16 cast, H^T copies, even-row output copies (+s_u bias)
  Vector:  U'^T scale-copies (w3*x+w1), s_u row-dots, odd-row output copies
  GpSimd:  U fp16 cast, identity
  Sync:    all HBM DMAs (queue): inputs + 16x 512KiB output writes

Sharding: data-parallel over batch - 8 batches, one per NeuronCore.
Self-contained: hardcodes shapes B=8, T=J=2048, D=128.
"""

import numpy as np

import concourse.mybir as mybir
import concourse.tile as tile
from concourse import bacc
from concourse.bass import broadcast_tensor_aps
from concourse.bass_utils import run_bass_kernel_spmd
from concourse.masks import make_identity

F32 = mybir.dt.float32
F16 = mybir.dt.float16
IDENT = mybir.ActivationFunctionType.Identity
MULT = mybir.AluOpType.mult
ADD = mybir.AluOpType.add

B = 8          # batch -> one per core
T = 2048       # rows of S (t) and columns (j)
D = 128        # feature dim = contraction K
P = 128        # partitions / tile edge
NT = T // P    # 16 tiles per side
MMW = 512      # matmul moving width (1 PSUM bank of f32 out)
NH = T // MMW  # 4 chunks per output row-block

U_LEAD = 3     # U-transpose lead distance ahead of the main loop

_NC_CACHE = {}


def _build_nc():
    nc = bacc.Bacc(
        "TRN2",
        target_bir_lowering=False,
        debug=False,
        num_devices=B,
    )
    H = nc.dram_tensor("H", [T, D], F32, kind="ExternalInput").ap()
    U = nc.dram_tensor("U", [T, D], F32, kind="ExternalInput").ap()
    w = nc.dram_tensor("weight", [3 * D], F32, kind="ExternalInput").ap()
    # Holds S^T (fp16) for this batch; host transposes + upcasts on gather.
    S = nc.dram_tensor("S", [T, T], F16, kind="ExternalOutput").ap()

    with tile.TileContext(nc) as tc:
        with (
            tc.tile_pool(name="persist", bufs=1) as pp,
            tc.tile_pool(name="psum_mm", bufs=3, space="PSUM") as psum_mm,
            tc.tile_pool(name="psum_tr", bufs=2, space="PSUM") as psum_tr,
            tc.tile_pool(name="outp", bufs=6) as outp,
        ):
            # weight DMA first on the gpsimd queue (before make_identity's
            # gpsimd work) -- it gates the w-prep matmuls.
            w_row = pp.tile([1, 3 * D], F32)
            nc.gpsimd.dma_start(out=w_row[:], in_=w.unsqueeze(0))
            ident16 = pp.tile([P, P], F16)
            make_identity(nc, ident16[:])

            # Inputs, natural layout [p, ti, d] (t = ti*128 + p) for H;
            # U loads p-major (fully contiguous per partition -> fast DMA).
            # U_sb[p, k, d] = U[16p + k, d]: j-tile k covers j = 16q + k,
            # a row permutation absorbed by the output DMA access pattern.
            H_sb = pp.tile([P, NT, D], F32)
            U_sb = pp.tile([P, NT, D], F32)
            H16 = pp.tile([P, NT, D], F16)
            U16 = pp.tile([P, NT, D], F16)
            H_r = H.rearrange("(ti p) d -> p ti d", p=P)
            U_r = U.rearrange("(p k) d -> p k d", p=P)
            # H leads (it gates the H transposes -> the first matmuls);
            # U's first quarter early so U transposes k=0..2 can start.
            nc.sync.dma_start(out=H_sb[:, 0:4, :], in_=H_r[:, 0:4, :])
            nc.sync.dma_start(out=H_sb[:, 4:8, :], in_=H_r[:, 4:8, :])
            nc.sync.dma_start(out=U_sb[:, 0:4, :], in_=U_r[:, 0:4, :])
            nc.sync.dma_start(out=H_sb[:, 8:12, :], in_=H_r[:, 8:12, :])
            nc.sync.dma_start(out=H_sb[:, 12:16, :], in_=H_r[:, 12:16, :])
            nc.sync.dma_start(out=U_sb[:, 4:8, :], in_=U_r[:, 4:8, :])
            nc.sync.dma_start(out=U_sb[:, 8:16, :], in_=U_r[:, 8:16, :])

            one_cell = pp.tile([1, 1], F32)
            nc.vector.memset(one_cell[:], 1.0)
            ones_row = pp.tile([1, P], F32)
            nc.vector.memset(ones_row[:], 1.0)

            # w1/w3 columns [d, 1] via rank-1 matmuls (w_row_chunk^T x 1)
            wc_b = psum_mm.tile([P, 2 * MMW], F32, tag="mm", name="wc_b")
            wcol_ps = wc_b[:, 0:2]
            for i, k in enumerate((0, 2)):
                nc.tensor.matmul(
                    wcol_ps[:, i : i + 1],
                    w_row[0:1, k * D : (k + 1) * D],
                    one_cell[:],
                    start=True,
                    stop=True,
                )
            wcol = pp.tile([P, 2], F32)
            nc.scalar.copy(wcol[:], wcol_ps[:])
            w1col = wcol[:, 0:1]
            w3col = wcol[:, 1:2]
            # w2 broadcast to all partitions: w2b[p, d] = w2[d], via
            # ones-column (K=1) matmul. Feeds the s_u row-dots on DVE.
            w2_b = psum_mm.tile([P, 2 * MMW], F32, tag="mm", name="w2_b")
            w2_ps = w2_b[:, 0:D]
            nc.tensor.matmul(
                w2_ps[:], ones_row[:], w_row[0:1, D : 2 * D], start=True, stop=True
            )
            w2b = pp.tile([P, D], F32)
            nc.scalar.copy(w2b[:], w2_ps[:])

            # fp16 casts, chunked so transposes can start as DMAs land.
            # H on scalar (gates transposes), U on gpsimd (otherwise idle).
            def cast_h(c):
                csl = slice(4 * c, 4 * c + 4)
                nc.scalar.copy(H16[:, csl, :], H_sb[:, csl, :])

            def cast_u(c):
                csl = slice(4 * c, 4 * c + 4)
                nc.gpsimd.tensor_copy(U16[:, csl, :], U_sb[:, csl, :])

            # Persistent transposed operands (d on partitions), fp16
            HT = pp.tile([P, T], F16)      # H^T
            UpT = pp.tile([P, T], F16)     # U'^T = w3 * U^T + w1
            s_u_col = pp.tile([P, NT], F32)  # s_u, one col per j-tile
            suprod = pp.tile([P, 4, D], F32)  # U * w2 scratch, one 4-chunk

            def do_h(ti):
                csl = slice(ti * P, (ti + 1) * P)
                h_ps = psum_tr.tile([P, P], F16, tag="tr", name=f"h_ps{ti}")
                nc.tensor.transpose(h_ps[:], H16[:, ti, :], ident16[:])
                nc.scalar.copy(HT[:, csl], h_ps[:])

            def do_u(k):
                csl = slice(k * P, (k + 1) * P)
                u_ps = psum_tr.tile([P, P], F16, tag="tr", name=f"u_ps{k}")
                nc.tensor.transpose(u_ps[:], U16[:, k, :], ident16[:])
                # U'^T chunk = w3 * U^T + w1, rounded to fp16
                nc.vector.tensor_scalar(
                    UpT[:, csl], u_ps[:], w3col, w1col, op0=MULT, op1=ADD
                )

            def do_su(c):
                # s_u[16p+k] = sum_d U_sb[p,k,d] * w2[d] for k in one
                # 4-tile chunk, on DVE: broadcast-multiply then reduce-X.
                csl = slice(4 * c, 4 * c + 4)
                in0 = U_sb[:, csl, :]
                in1 = w2b[:].unsqueeze(1)  # [P, 1, D] -> bcast over k
                in0b, in1b = broadcast_tensor_aps(in0, in1)
                nc.vector.tensor_tensor(
                    out=suprod[:], in0=in0b, in1=in1b, op=MULT
                )
                nc.vector.tensor_reduce(
                    s_u_col[:, csl], suprod[:], axis=mybir.AxisListType.X, op=ADD
                )

            cast_h(0)
            cast_u(0)
            for ti in range(4):
                do_h(ti)
            for k in range(U_LEAD):
                do_u(k)
            do_su(0)
            cast_h(1)
            cast_u(1)

            # Main loop: one 128-row output block of S^T per jt. Four N=512
            # fp16 matmuls land pairwise in 2-bank [128,1024] PSUM tiles,
            # each pair drained by ONE 1024-wide copy (halves the copy
            # instruction count -- per-instruction overhead dominates).
            # The copy folds the s_u bias and the fp16 downcast. Copies of
            # a row-block go to ONE engine so each output DMA waits on a
            # single engine's semaphore; scalar (faster) takes the even
            # rows + the last, vector the remaining odd rows; the last
            # row's two copies split across both engines to cut the tail.
            # H transposes/casts fold lazily into jt==0.
            for jt in range(NT):
                if 1 <= jt <= 3:
                    do_su(jt)
                if jt + U_LEAD < NT:
                    do_u(jt + U_LEAD)
                jsl = slice(jt * P, (jt + 1) * P)
                S_rows = S.rearrange("(q s) t -> s q t", s=NT)[jt]
                su_b = s_u_col[:, jt : jt + 1]
                out_sb = outp.tile([P, T], F16)
                for half in range(2):
                    osl = slice(half * 2 * MMW, (half + 1) * 2 * MMW)
                    ps = psum_mm.tile(
                        [P, 2 * MMW], F32, tag="mm", name=f"mm{jt}_{half}"
                    )
                    for q in range(2):
                        h = 2 * half + q
                        if jt == 0 and h >= 1:
                            if h + 1 < 4:
                                cast_h(h + 1)
                            for ti in range(4 * h, 4 * h + 4):
                                do_h(ti)
                        if jt == 1 and h < 2:
                            cast_u(h + 2)
                        nc.tensor.matmul(
                            ps[:, q * MMW : (q + 1) * MMW],
                            UpT[:, jsl],
                            HT[:, h * MMW : (h + 1) * MMW],
                            start=True,
                            stop=True,
                        )
                    on_scalar = jt % 2 == 0 or (jt == NT - 1 and half == 0)
                    if on_scalar:
                        nc.scalar.activation(
                            out_sb[:, osl], ps[:], IDENT, bias=su_b, scale=1.0
                        )
                    else:
                        nc.vector.tensor_scalar_add(out_sb[:, osl], ps[:], su_b)
                    # First row-block: halves so output DMA starts early.
                    if jt == 0:
                        nc.sync.dma_start(out=S_rows[:, osl], in_=out_sb[:, osl])
                if jt > 0:
                    nc.sync.dma_start(out=S_rows[:, :], in_=out_sb[:])

    nc.compile()
    return nc


def _get_nc():
    if "nc" not in _NC_CACHE:
        _NC_CACHE["nc"] = _build_nc()
    return _NC_CACHE["nc"]


def kernel_with_results(H, U, weight, trace=False):
    assert H.shape == (B, T, D) and U.shape == (B, T, D)
    assert weight.shape == (3 * D,)
    nc = _get_nc()
    in_maps = [
        {
            "H": np.ascontiguousarray(H[b], dtype=np.float32),
            "U": np.ascontiguousarray(U[b], dtype=np.float32),
            "weight": np.ascontiguousarray(weight, dtype=np.float32),
        }
        for b in range(B)
    ]
    res = run_bass_kernel_spmd(nc, in_maps, list(range(B)), trace=trace)
    # device output is S^T (fp16) per batch
    out = np.stack(
        [np.asarray(res.results[b]["S"]).T.astype(np.float32) for b in range(B)],
        axis=0,
    )
    return out, res


def kernel(H, U, weight):
    out, _ = kernel_with_results(H, U, weight)
    return out


if __name__ == "__main__":
    rng = np.random.default_rng(0)
    H = rng.standard_normal((B, T, D)).astype(np.float32)
    U = rng.standard_normal((B, T, D)).astype(np.float32)
    w = rng.random(3 * D).astype(np.float32)
    out = kernel(H, U, w)
    print(out.shape, out.dtype)
